# revision 8
# baseline (speedup 1.0000x reference)
"""Trainium2 Bass kernel for the CubeSimulator problem.

Reference computation (shapes): rotate (96,96,96) grids, build a per-voxel
line-of-sight velocity u and intensity I = exp(L), then a Gaussian-KDE cube
cube[i,j,v] = norm * sum_z exp(-(vel_v - u)^2/sig^2) * I, followed by a
"trilinear" downsample (96,96,64) -> (32,64,64).

Key exact simplifications (validated against the reference in fp32,
rel err ~6e-6):
 - downsample axis0 (96->32, scale 3): output coords land exactly on
   integers 3k+1, so it is a pure row selection -> only 32 of 96 i-rows
   are ever needed (3x less KDE work).
 - downsample axis2 (64->64) is exactly the identity.
 - downsample axis1 (96->64) is an exact 2-tap stencil with weights
   0.75/0.25 (even) / 0.25/0.75 (odd), applied as one TensorE matmul over
   the j partition axis.
 - exp(L - (vel_v-u)^2/sig^2) = exp(A + vel_v*B + c_v) with
   A = L + ln(norm) - u^2/sig^2, B = 2u/sig^2, c_v = -vel_v^2/sig^2;
   A and B are precomputed per voxel.
 - tanh(r/2)/r = (e^r - 1)/(r (e^r + 1)) evaluated with a single
   reciprocal; r = exp(0.5 ln(max(q,1e-35))) keeps every activation in
   the natural_log_exp_and_others table set (one ACT table load) and
   avoids the loose-tolerance Sqrt.

Per velocity bin, two engine-balanced paths (split tuned on the
instruction cost model):
 - affine path: VectorE tensor_scalar (B*vv + c_v), V/G tensor_add (+A),
   with KDE_VB bins batched into one wide ScalarE Exp.
 - factored path: exp(A + vv*B + c_v) = exp(A) * exp(vv*B + c_v) -- one
   ScalarE Exp (scale=vv immediate, bias=c_v per-partition AP) and one
   V/G multiply by P0 = exp(A).
The z-reduction is a per-(bin, i-row) TensorE matmul with the exp tile as
the stationary operand and a ones-vector moving, accumulating the cube as
[j=96 partitions, (i,v)] in PSUM, which makes the j-downsample a single
stationary-W matmul.

Sharding: the 32 needed i-rows are split 4-per-core across 8 cores (pure
data parallel over pixels); each core's device layout is [z=96 partitions,
pixels=4*96=384 free].  Runtime scalars (rotation trig, sigma, the 64
velocity values) are baked into the instruction stream as immediates since
the kernel is compiled per call.
"""

import math

import numpy as np

import concourse.bacc as bacc
import concourse.bass as bass
import concourse.mybir as mybir
import concourse.tile as tile
from concourse.bass_utils import run_bass_kernel_spmd

G = 96            # up_gal grid size
NV = 64           # velocity bins
N_CORES = 8
OUT_I = 32        # downsampled i rows (= VEL_RES in the reference's axis naming)
ROWS_PER_CORE = OUT_I // N_CORES   # 4
PX = ROWS_PER_CORE * G             # 384 pixels per core
OUT_J = 64

F32 = mybir.dt.float32
AF = mybir.ActivationFunctionType
OP = mybir.AluOpType

LAST_EXEC_NS = None  # filled in when run with BASS_TRACE=1

# tuning knobs (validated via TimelineSim sweeps)
KDE_VB = 8                   # velocity bins per group
KDE_FACT_SLOTS = (1, 3, 5, 7)  # slots per group using the factored path
KDE_NGC = 0.25               # fraction of affine-path adds routed to GpSimd
KDE_FACT_G = 0.75            # fraction of factored-path mults routed to GpSimd
ABLATE = set()         # {'mm','tt','ts','exp'} - sim-only ablation switches


def _build_program(ci, si, cr, sr, sig2, lnnorm, vel, fact_slots=None):
    if fact_slots is None:
        fact_slots = KDE_FACT_SLOTS
    nc = bacc.Bacc("TRN2")

    xs = nc.dram_tensor("xs", [G, PX], F32, kind="ExternalInput")
    ys = nc.dram_tensor("ys", [G, PX], F32, kind="ExternalInput")
    zs = nc.dram_tensor("zs", [G, PX], F32, kind="ExternalInput")
    # per-velocity-bin exp biases c_v = -vel_v^2/sig^2 (replicated across
    # partitions; used as per-partition bias APs on factored-path Exps)
    bc = nc.dram_tensor("bc", [128, NV], F32, kind="ExternalInput")
    # j-downsample stencil matrix (96 -> 64, 2 taps per output)
    wj = nc.dram_tensor("wj", [G, OUT_J], F32, kind="ExternalInput")
    # identity for PE transposes of the [v, (i,j)] psum cube
    eye = nc.dram_tensor("eye", [NV, NV], F32, kind="ExternalInput")
    # one-hot stationary bank (float32r: PE-side reduced-precision fp32)
    ohd = nc.dram_tensor("ohd", [G, 3 * NV], mybir.dt.float32r,
                         kind="ExternalInput")
    out = nc.dram_tensor("out", [OUT_J, ROWS_PER_CORE * NV], F32,
                         kind="ExternalOutput")

    with tile.TileContext(nc) as tc:
        with (
            tc.tile_pool(name="io", bufs=1) as io,
            tc.tile_pool(name="prep", bufs=1) as prep,
            tc.tile_pool(name="kde", bufs=2) as kde,
            tc.tile_pool(name="psum", bufs=1, space="PSUM") as psum,
        ):
            xt = io.tile([G, PX], F32, tag="xt")
            yt = io.tile([G, PX], F32, tag="yt")
            zt = io.tile([G, PX], F32, tag="zt")
            nc.sync.dma_start(out=xt[:], in_=xs[:])
            nc.sync.dma_start(out=yt[:], in_=ys[:])
            nc.sync.dma_start(out=zt[:], in_=zs[:])
            wjt = io.tile([G, OUT_J], F32, tag="wjt")
            nc.sync.dma_start(out=wjt[:], in_=wj[:])
            eyet = io.tile([NV, NV], F32, tag="eyet")
            nc.sync.dma_start(out=eyet[:], in_=eye[:])
            bct = io.tile([128, NV], F32, tag="bct")
            nc.sync.dma_start(out=bct[:], in_=bc[:])

            def vtile(name):
                return prep.tile([G, PX], F32, tag=name, name=name)

            # Prep. Only tensor_scalar / tensor_tensor / activation are used
            # -- the S2S2D2_STT (scalar_tensor_tensor) ISA struct has a
            # single sync-wait slot and cannot be scheduled where Tile needs
            # multiple waits.
            # Rotated coordinates (R = Rx(inc) @ Rz(rot)); the rx/ry legs run
            # on VectorE (critical path), the rz/intensity leg on GpSimd.
            xa, ya, rx = vtile("xa"), vtile("ya"), vtile("rx")
            xb, yb, t3 = vtile("xb"), vtile("yb"), vtile("t3")
            za, ry = vtile("za"), vtile("ry")
            nc.vector.tensor_scalar_mul(xa[:], xt[:], cr)
            nc.vector.tensor_scalar_mul(ya[:], yt[:], -sr)
            nc.vector.tensor_add(rx[:], xa[:], ya[:])
            nc.vector.tensor_scalar_mul(xb[:], xt[:], ci * sr)
            nc.vector.tensor_scalar_mul(yb[:], yt[:], ci * cr)
            nc.vector.tensor_add(t3[:], xb[:], yb[:])
            nc.vector.tensor_scalar_mul(za[:], zt[:], -si)
            nc.vector.tensor_add(ry[:], t3[:], za[:])
            xc, yc, t5 = vtile("xc"), vtile("yc"), vtile("t5")
            zb, rz = vtile("zb"), vtile("rz")
            nc.gpsimd.tensor_scalar_mul(xc[:], xt[:], si * sr)
            nc.gpsimd.tensor_scalar_mul(yc[:], yt[:], si * cr)
            nc.gpsimd.tensor_add(t5[:], xc[:], yc[:])
            nc.gpsimd.tensor_scalar_mul(zb[:], zt[:], ci)
            nc.gpsimd.tensor_add(rz[:], t5[:], zb[:])

            # in-plane radius r via exp(0.5*ln(q)) -- avoids the loose-
            # tolerance Sqrt activation.  q is clamped away from 0 once so
            # every division below is finite (r >= 3e-18).
            sqx, sqy, q, qs = vtile("sqx"), vtile("sqy"), vtile("q"), vtile("qs")
            lnq, r = vtile("lnq"), vtile("r")
            nc.scalar.activation(sqx[:], rx[:], AF.Square)
            nc.vector.tensor_mul(sqy[:], ry[:], ry[:])
            nc.vector.tensor_add(q[:], sqy[:], sqx[:])
            nc.vector.tensor_scalar_max(qs[:], q[:], 1e-35)
            nc.scalar.activation(lnq[:], qs[:], AF.Ln)
            nc.scalar.activation(r[:], lnq[:], AF.Exp, scale=0.5)

            # u0 = rx*tanh(r/2)/r computed as rx*(e^r-1) / (r*(e^r+1)) --
            # one reciprocal, and every activation stays in the
            # natural_log_exp_and_others table set (single table load).
            # The -200*si amplitude folds into the s1/Bt scales below.
            er, ed = vtile("er"), vtile("ed")
            den, rec, num = vtile("den"), vtile("rec"), vtile("num")
            t1, u0 = vtile("t1"), vtile("u0")
            nc.scalar.activation(er[:], r[:], AF.Exp)
            nc.vector.tensor_scalar_add(ed[:], er[:], 1.0)
            nc.vector.tensor_mul(den[:], ed[:], r[:])
            nc.vector.reciprocal(rec[:], den[:])
            nc.vector.tensor_scalar_add(num[:], er[:], -1.0)
            nc.vector.tensor_mul(t1[:], rx[:], num[:])
            nc.vector.tensor_mul(u0[:], t1[:], rec[:])

            # A = L + lnnorm - (u/sig)^2 ; L = -r/3 - 2|rz| ; B = 2u/sig^2
            az, azs, rterm, Lt = (vtile("az"), vtile("azs"), vtile("rterm"),
                                  vtile("Lt"))
            s1, ssq, At, Bt, P0t = (vtile("s1"), vtile("ssq"), vtile("At"),
                                    vtile("Bt"), vtile("P0t"))
            nc.scalar.activation(az[:], rz[:], AF.Abs)
            nc.gpsimd.tensor_scalar_mul(azs[:], az[:], -2.0)
            nc.gpsimd.tensor_scalar(rterm[:], r[:], -1.0 / 3.0, lnnorm,
                                    OP.mult, OP.add)
            nc.gpsimd.tensor_add(Lt[:], azs[:], rterm[:])
            usc = -200.0 * si
            nc.vector.tensor_scalar_mul(s1[:], u0[:], usc / math.sqrt(sig2))
            nc.scalar.activation(ssq[:], s1[:], AF.Square)
            nc.vector.tensor_sub(At[:], Lt[:], ssq[:])
            nc.vector.tensor_scalar_mul(Bt[:], u0[:], usc * 2.0 / sig2)
            nc.scalar.activation(P0t[:], At[:], AF.Exp)

            # one-hot stationary bank: oh[:, 2*NV-iv : 3*NV-iv] is a [G, NV]
            # matrix whose column iv is all-ones and the rest zeros, so one
            # accumulating matmul per bin lands row iv of cube_z = sum_z F_iv
            # while leaving the other rows untouched.
            FR = mybir.dt.float32r
            oh = io.tile([G, 3 * NV], FR, tag="oh")
            nc.sync.dma_start(out=oh[:], in_=ohd[:])

            # cube_z[v, px] = sum_z exp-term   (v on partitions)
            cube_z = psum.tile([NV, PX], F32)
            n_mm = [0]

            def reduce_bin(iv, src_ap):
                nc.tensor.matmul(cube_z[:, :],
                                 oh[:, 2 * NV - iv:3 * NV - iv], src_ap,
                                 start=(n_mm[0] == 0), stop=(n_mm[0] == NV - 1))
                n_mm[0] += 1

            # Two paths per velocity bin, mixed to balance engines:
            #  - affine path: arg = (B*vv + c_v) + A  (VectorE ts + V/G tt),
            #    VB-batched into one wide ScalarE Exp.
            #  - factored path: exp(A + vv*B + c_v) = P0 * exp(vv*B + c_v)
            #    (one ScalarE Exp with scale/bias immediates + one V/G mult;
            #    exact to fp32 rounding since both factors are exp outputs).
            VB = KDE_VB
            for g in range(NV // VB):
                bins = list(range(g * VB, (g + 1) * VB))
                cbins = [b for b in bins if (b % VB) not in fact_slots]
                fbins = [b for b in bins if (b % VB) in fact_slots]
                srcs = {}
                # factored-path bins first: their Exps depend only on Bt, so
                # ScalarE starts each group without stalling on the affine
                # arg builds (Tile priority follows emission order)
                nfb = len(fbins)
                for k, iv in enumerate(fbins):
                    vv = float(vel[iv])
                    e1 = kde.tile([G, PX], F32, tag="e1", bufs=4)
                    nc.scalar.activation(e1[:], Bt[:], AF.Exp, scale=vv,
                                         bias=bct[0:G, iv:iv + 1])
                    m1 = kde.tile([G, PX], FR, tag="m1", bufs=4)
                    eng = nc.gpsimd if k < KDE_FACT_G * nfb else nc.vector
                    eng.tensor_mul(m1[:], e1[:], P0t[:])
                    srcs[iv] = (m1, 0)
                ncb = len(cbins)
                if ncb:
                    argw = kde.tile([G, ncb * PX], F32, tag="argw")
                    tmpw = kde.tile([G, ncb * PX], F32, tag="tmpw")
                    for k, iv in enumerate(cbins):
                        vv = float(vel[iv])
                        cv = -vv * vv / sig2
                        sl = slice(k * PX, (k + 1) * PX)
                        nc.vector.tensor_scalar(tmpw[:, sl], Bt[:], vv, cv,
                                                OP.mult, OP.add)
                        eng = nc.gpsimd if k < KDE_NGC * ncb else nc.vector
                        eng.tensor_add(argw[:, sl], tmpw[:, sl], At[:])
                    exw = kde.tile([G, ncb * PX], FR, tag="exw")
                    nc.scalar.activation(exw[:], argw[:], AF.Exp)
                    for k, iv in enumerate(cbins):
                        srcs[iv] = (exw, k * PX)
                # reduce over z (partitions): one accumulating matmul per
                # bin lands row iv of cube_z.
                for iv in bins:
                    if 'mm' in ABLATE:
                        break
                    t, off0 = srcs[iv]
                    reduce_bin(iv, t[:, off0:off0 + PX])

            # tail: cube_z [v, (i,j)] -> transpose per i-row -> [j, (i,v)]
            # -> j-downsample matmul (stationary wj) -> out [jj, (i,v)]
            cz_sb = io.tile([NV, PX], F32, tag="cz_sb")
            nc.vector.tensor_copy(cz_sb[:], cube_z[:])
            cube_jp = psum.tile([G, ROWS_PER_CORE * NV], F32)
            for ii in range(ROWS_PER_CORE):
                nc.tensor.transpose(cube_jp[:, ii * NV:(ii + 1) * NV],
                                    cz_sb[:, ii * G:(ii + 1) * G],
                                    eyet[:])
            cube_sb = io.tile([G, ROWS_PER_CORE * NV], F32, tag="cube_sb")
            nc.vector.tensor_copy(cube_sb[:], cube_jp[:])
            out_ps = psum.tile([OUT_J, ROWS_PER_CORE * NV], F32)
            nc.tensor.matmul(out_ps[:], wjt[:], cube_sb[:],
                             start=True, stop=True)
            out_sb = io.tile([OUT_J, ROWS_PER_CORE * NV], F32, tag="out_sb")
            nc.vector.tensor_copy(out_sb[:], out_ps[:])
            nc.sync.dma_start(out=out[:], in_=out_sb[:])

    return nc


def kernel(**inputs):
    inc = float(np.asarray(inputs["inclination"]).reshape(-1)[0])
    rot = float(np.asarray(inputs["sky_rot"]).reshape(-1)[0])
    lb = float(np.asarray(inputs["line_broadening"]).reshape(-1)[0])
    vel = np.asarray(inputs["velocity_grid"], np.float32).reshape(-1)
    X = np.asarray(inputs["Xgrid"], np.float32)
    Y = np.asarray(inputs["Ygrid"], np.float32)
    Z = np.asarray(inputs["Zgrid"], np.float32)

    ci, si = math.cos(inc), math.sin(inc)
    cr, sr = math.cos(rot), math.sin(rot)
    sig2 = float(np.float32(lb) * np.float32(lb))
    if not (sig2 > 0.0) or not math.isfinite(sig2):
        sig2 = 1e-30  # degenerate sigma: reference output is ~0/NaN anyway
    lnnorm = float(-0.5 * math.log(2.0 * math.pi * sig2))

    # The factored path computes exp(vv*B + c_v) whose argument is bounded by
    # u_max^2/sig^2 (u_max = 200*|sin(inc)| rigorously bounds |u|).  If that
    # could overflow fp32, fall back to the always-safe affine path (its
    # fused exponent is <= ln(norm)).
    umax2 = (200.0 * abs(si)) ** 2
    fact_slots = KDE_FACT_SLOTS if umax2 / sig2 <= 80.0 else ()
    nc = _build_program(ci, si, cr, sr, sig2, lnnorm, vel, fact_slots)
    nc.finalize()

    bcv = np.ascontiguousarray(
        np.tile((-(vel.astype(np.float64) ** 2) / sig2).astype(np.float32),
                (128, 1)))
    ohv = np.zeros((G, 3 * NV), np.float32)
    ohv[:, 2 * NV] = 1.0
    wjv = np.zeros((G, OUT_J), np.float32)
    for m in range(OUT_J // 2):
        wjv[3 * m, 2 * m] = 0.75
        wjv[3 * m + 1, 2 * m] = 0.25
        wjv[3 * m + 1, 2 * m + 1] = 0.25
        wjv[3 * m + 2, 2 * m + 1] = 0.75

    in_maps = []
    for c in range(N_CORES):
        rows = [3 * k + 1 for k in range(ROWS_PER_CORE * c,
                                         ROWS_PER_CORE * (c + 1))]
        def shard(a):
            s = a[rows]                        # (4, 96, 96) = (i, j, z)
            s = s.transpose(2, 0, 1).reshape(G, PX)   # [z, i*96+j]
            return np.ascontiguousarray(s)
        in_maps.append({"xs": shard(X), "ys": shard(Y), "zs": shard(Z),
                        "bc": bcv, "wj": wjv,
                        "eye": np.eye(NV, dtype=np.float32),
                        "ohd": ohv})

    res = run_bass_kernel_spmd(nc, in_maps, core_ids=list(range(N_CORES)))
    global LAST_EXEC_NS
    LAST_EXEC_NS = res.exec_time_ns

    parts = []
    for c in range(N_CORES):
        o = res.results[c]["out"]              # (64, 256) = [jj, i*64+v]
        parts.append(o.reshape(OUT_J, ROWS_PER_CORE, NV).transpose(1, 0, 2))
    return np.concatenate(parts, axis=0).astype(np.float32)  # (32, 64, 64)

